# revision 13
# baseline (speedup 1.0000x reference)
"""Multi-resolution 1D ROI max-pooling kernel for Trainium2 (raw Bass).

Reference computation: x[4096, 16384] f32; for each pool width p in
[1, 2, 4, 8, 16] max-pool the W dim into p equal bins (16384 % 16 == 0 so
all bins are exact), concatenate -> out[4096, 31] with column layout
[m1 | m2(2) | m4(4) | m8(8) | m16(16)].

Strategy: pure data parallel over the batch dim -- 8 cores x 512 rows.
Per core, rows are processed as 4 row-tiles of 128 partitions; each
row-tile's 16384 columns stream in as 4 chunks of [128, 4096] (2 MiB DMAs,
multi-buffered) so the DVE can reduce chunks as they land and only the last
small chunk's reduce is exposed after the final DMA byte. The 16 finest
bins (1024 wide) come from segmented reduce_max per chunk; coarser levels
(8/4/2/1 bins) are a cascade of tiny pairwise reduces (max is hierarchical).
All results accumulate in one small SBUF tile, flushed with one DMA.

Raw Bass (not Tile): every cross-engine dependency is a standalone wait_ge
on the issuing engine's queue, since this toolchain's static-DMA lowering
rejects DMA instructions with more than one embedded sync wait. DVE ops do
not interlock with each other, so same-engine RAW hazards also need waits.
"""

from contextlib import ExitStack

import numpy as np

from concourse import bass, mybir
from concourse.bass_utils import run_bass_kernel_spmd

N_CORES = 8
B, W = 4096, 16384
ROWS = B // N_CORES   # 512 rows per core
P = 128               # SBUF partitions
NT = ROWS // P        # 4 row-tiles per core
NBINS = 16
BIN_W = W // NBINS    # 1024
OUT_COLS = 31         # 1 + 2 + 4 + 8 + 16
CW = 4096             # chunk width (columns per DMA)
NCPT = W // CW        # chunks per row-tile
BPC = CW // BIN_W     # bins per chunk
NBUF = 6              # in-flight chunk buffers

_nc_cache = None


def _build_kernel(repeat: int = 1, serialize: bool = False):
    """Build the per-core Bass program.

    repeat > 1 re-runs the whole per-core workload that many times inside
    one NEFF (reading the same input rows) -- used only for timing.
    serialize=True gates each repeat's first load on the previous repeat's
    store completion, so repeats cannot overlap and the timing slope equals
    the true single-shot kernel time (ramp + tail included).
    """
    nc = bass.Bass()
    x = nc.declare_dram_parameter("x", [ROWS, W], mybir.dt.float32, isOutput=False)
    out = nc.declare_dram_parameter(
        "out", [ROWS, OUT_COLS], mybir.dt.float32, isOutput=True
    )

    # DVE program order is fixed; precompute the vs (DVE-progress sem) value
    # after each global chunk-reduce so the load loop can emit WAR waits.
    # Per repeat r, per tile i: NCPT chunk-reduces then 4 cascade reduces.
    vs_after_reduce = {}  # global chunk index -> vs value after its reduce
    vs = 0
    for r in range(repeat):
        for i in range(NT):
            for j in range(NCPT):
                vs += 1
                vs_after_reduce[(r * NT + i) * NCPT + j] = vs
            vs += 4  # cascade reduces
    vs_per_repeat = vs // repeat

    with (
        ExitStack() as ctx,
        nc.Block() as block,
    ):
        slots = [
            ctx.enter_context(
                nc.sbuf_tensor(f"xt{s}", [P, CW], mybir.dt.float32)
            )
            for s in range(NBUF)
        ]
        res = ctx.enter_context(
            nc.sbuf_tensor("res", [P, NT * OUT_COLS], mybir.dt.float32)
        )
        ld = [ctx.enter_context(nc.semaphore(f"ld{s}")) for s in range(NBUF)]
        st = ctx.enter_context(nc.semaphore("st"))
        vsm = ctx.enter_context(nc.semaphore("vs"))

        NCHUNKS = repeat * NT * NCPT

        def emit_store(gpsimd, r):
            gpsimd.wait_ge(vsm, vs_per_repeat * (r + 1))
            gpsimd.dma_start(
                out[:].rearrange("(n p) c -> p n c", p=P),
                res[:].rearrange("p (n c) -> p n c", n=NT),
            ).then_inc(st, 16)

        @block.gpsimd
        def _(gpsimd):
            n_stores = 0
            for g in range(NCHUNKS):
                r, rem = divmod(g, NT * NCPT)
                i, j = divmod(rem, NCPT)
                if serialize and rem == 0 and r > 0:
                    # Full serialization between repeats (timing mode): the
                    # previous repeat's store is emitted below before this
                    # wait in program order, and must complete first.
                    emit_store(gpsimd, r - 1)
                    n_stores += 1
                    gpsimd.wait_ge(st, 16 * r)
                if g >= NBUF:
                    # WAR: the slot's previous chunk must have been consumed
                    # by its reduce before the DMA may overwrite it.
                    gpsimd.wait_ge(vsm, vs_after_reduce[g - NBUF])
                gpsimd.dma_start(
                    slots[g % NBUF][:, :],
                    x[(i % NT) * P : (i % NT + 1) * P, j * CW : (j + 1) * CW],
                ).then_inc(ld[g % NBUF], 16)
            # Last (or, in non-serialized mode, only) store.
            emit_store(gpsimd, repeat - 1)
            n_stores += 1
            gpsimd.wait_ge(st, 16 * n_stores)

        @block.vector
        def _(vector):
            nvs = 0
            for r in range(repeat):
                if serialize and r > 0:
                    # WAR: don't overwrite res while repeat r-1's store reads.
                    vector.wait_ge(st, 16 * r)
                for i in range(NT):
                    o = res[:, i * OUT_COLS : (i + 1) * OUT_COLS]
                    for j in range(NCPT):
                        g = (r * NT + i) * NCPT + j
                        vector.wait_ge(ld[g % NBUF], 16 * (g // NBUF + 1))
                        vector.reduce_max(
                            o[:, 15 + j * BPC : 15 + (j + 1) * BPC],
                            slots[g % NBUF][:, :].rearrange(
                                "p (b w) -> p b w", b=BPC
                            ),
                            axis=mybir.AxisListType.X,
                        ).then_inc(vsm, 1)
                        nvs += 1
                    lo, size = 15, 16
                    while size > 1:
                        size //= 2
                        # DVE ops don't interlock; wait for the previous
                        # level's writes to land before reading them.
                        vector.wait_ge(vsm, nvs)
                        vector.reduce_max(
                            o[:, lo - size : lo],
                            o[:, lo : lo + 2 * size].rearrange(
                                "p (b t) -> p b t", t=2
                            ),
                            axis=mybir.AxisListType.X,
                        ).then_inc(vsm, 1)
                        nvs += 1
                        lo -= size

    return nc


def kernel(x: np.ndarray) -> np.ndarray:
    global _nc_cache
    if _nc_cache is None:
        _nc_cache = _build_kernel()
    nc = _nc_cache

    x = np.ascontiguousarray(x, dtype=np.float32)
    in_maps = [{"x": x[c * ROWS : (c + 1) * ROWS]} for c in range(N_CORES)]
    res = run_bass_kernel_spmd(nc, in_maps, core_ids=list(range(N_CORES)))
    return np.concatenate(
        [res.results[c]["out"] for c in range(N_CORES)], axis=0
    )
